# revision 51
# baseline (speedup 1.0000x reference)
"""ALiBi bias application on 8 TRN2 NeuronCores.

out[b,h,i,j] = scores[b,h,i,j] - slope_h * (pos[b,i] - pos[b,j])
             = (scores[b,h,i,j] - slope_h*pos[b,i]) + slope_h*pos[b,j]

Pure streaming elementwise problem (memory-bound). Sharding: flatten
(B,H) -> 32 matrices, core c owns contiguous matrices [4c, 4c+4) —
bias terms are fully local, no collectives. The tiny per-(b,h) bias
vectors (slope*pos) are precomputed on host and laid out to match the
on-device tile mapping; on device each element is touched by exactly
one fused VectorEngine op (scalar_tensor_tensor) between two big DMAs.
The column-bias row is shipped unreplicated (32 KB) and broadcast
across partitions on the idle TensorEngine (e0-weights matmul into
PSUM, DVE copy to SBUF) instead of pushing a 128x-replicated 4 MiB
tensor through the saturated DMA engines.

Measured: ~325 us on silicon (uncontended) — DMA engines ~313 us busy
at their 27 GB/s line rate for the 128 MiB/core of scores+out traffic,
i.e. ~96% of the hardware floor.
"""

import sys

if "/opt/trn_rl_repo" not in sys.path:
    sys.path.insert(0, "/opt/trn_rl_repo")

import numpy as np

import concourse.bacc as bacc
import concourse.bass as bass
import concourse.mybir as mybir
from concourse.bass_utils import run_bass_kernel_spmd
from concourse.tile import TileContext

B, H, S = 2, 16, 2048
NCORES = 8
M_PER_CORE = (B * H) // NCORES  # 4 matrices per core
ROWS_PER_CHUNK = 512  # contiguous DRAM rows per DMA chunk (4 MiB)
DATA_BUFS = 4
K_SUB = ROWS_PER_CHUNK // 128  # rows owned by one partition per chunk
CHUNKS_PER_MAT = S // ROWS_PER_CHUNK
N_CHUNKS = M_PER_CORE * CHUNKS_PER_MAT
FREE = K_SUB * S  # SBUF free-dim elems per partition per chunk

_F32 = mybir.dt.float32


def _build_graph(mode="pebcast3"):
    nc = bacc.Bacc()
    scores_ext = nc.declare_dram_parameter(
        "scores", [M_PER_CORE, S, S], _F32, isOutput=False
    )
    if mode == "packed":
        # colb ([128, M_PER_CORE*S]) and rowv ([128, N_CHUNKS*K_SUB]) packed
        # side by side — a single DMA/semaphore keeps downstream compute ops
        # within the per-instruction sync-wait limit (1 wait per instruction).
        bias_ext = nc.declare_dram_parameter(
            "bias", [128, M_PER_CORE * S + N_CHUNKS * K_SUB], _F32, isOutput=False
        )
    else:  # pebcast: rowv only; colv shipped unreplicated, broadcast via PE
        bias_ext = nc.declare_dram_parameter(
            "bias", [128, N_CHUNKS * K_SUB], _F32, isOutput=False
        )
        colv_ext = nc.declare_dram_parameter(
            "colv", [M_PER_CORE * S], _F32, isOutput=False
        )
    out_ext = nc.declare_dram_parameter("out", [M_PER_CORE, S, S], _F32, isOutput=True)
    ROW0 = M_PER_CORE * S if mode == "packed" else 0

    data_bufs = 5 if mode == "pebcast2" else DATA_BUFS
    # pebcast2: tiny const DMAs on the (start-idle) scalar ring so chunk0's
    # descriptors hit the sync ring immediately
    const_eng = nc.scalar if mode == "pebcast2" else nc.sync
    # pebcast3: emit the first data_bufs chunk loads BEFORE the const
    # prologue — program order sets Tile's priority, so chunk0's 4 MiB
    # spray (which feeds all 16 SDMA engines) leads the sync-ring FIFO
    # instead of trailing five tiny const DMAs
    n_pre = data_bufs if mode == "pebcast3" else 0

    with TileContext(nc) as tc:
        with (
            tc.tile_pool(name="const", bufs=1) as cpool,
            tc.tile_pool(name="data", bufs=data_bufs) as dpool,
        ):
            pre_tiles = {}
            for c in range(n_pre):
                m = c // CHUNKS_PER_MAT
                r0 = (c % CHUNKS_PER_MAT) * ROWS_PER_CHUNK
                t = dpool.tile([128, FREE], _F32, name="t", tag="t")
                nc.sync.dma_start(
                    out=t[:], in_=scores_ext[m, r0 : r0 + ROWS_PER_CHUNK, :]
                )
                pre_tiles[c] = t
            if mode == "packed":
                bias_sb = cpool.tile(
                    [128, M_PER_CORE * S + N_CHUNKS * K_SUB], _F32
                )
                colb_sb = bias_sb
                nc.sync.dma_start(out=bias_sb[:], in_=bias_ext[:])
            else:
                bias_sb = cpool.tile([128, N_CHUNKS * K_SUB], _F32)
                colb_sb = cpool.tile([128, M_PER_CORE * S], _F32)
                lhsT_sb = cpool.tile([128, 128], _F32)
                scratch = cpool.tile([128, S], _F32)
                const_eng.dma_start(out=bias_sb[:], in_=bias_ext[:])
                # e0 weights: out[p,f] = sum_k lhsT[k,p]*rhs[k,f] = rhs[0,f]
                nc.vector.memset(lhsT_sb[:], 0.0)
                nc.vector.memset(lhsT_sb[0:1, :], 1.0)
                nc.vector.memset(scratch[:], 0.0)
                psum_cols = S if mode == "pebcast2" else 512
                with tc.tile_pool(
                    name="psum", bufs=4 if mode == "pebcast" else 2,
                    space=bass.MemorySpace.PSUM,
                ) as ppool:
                    for m in range(M_PER_CORE):
                        # colv_m -> scratch row 0 (rows 1-127 stay zero)
                        const_eng.dma_start(
                            out=scratch[0:1, :],
                            in_=colv_ext[m * S : (m + 1) * S],
                        )
                        pt = None
                        for j in range(S // 512):
                            if j % (psum_cols // 512) == 0:
                                pt = ppool.tile([128, psum_cols], _F32)
                            jj = j % (psum_cols // 512)
                            nc.tensor.matmul(
                                pt[:, jj * 512 : (jj + 1) * 512],
                                lhsT_sb[:],
                                scratch[:, j * 512 : (j + 1) * 512],
                            )
                            if jj == psum_cols // 512 - 1:
                                off = m * S + (j + 1) * 512 - psum_cols
                                nc.vector.tensor_copy(
                                    colb_sb[:, off : off + psum_cols], pt[:]
                                )
            for c in range(N_CHUNKS):
                m = c // CHUNKS_PER_MAT
                r0 = (c % CHUNKS_PER_MAT) * ROWS_PER_CHUNK
                if c in pre_tiles:
                    t = pre_tiles[c]
                else:
                    t = dpool.tile([128, FREE], _F32, name="t", tag="t")
                    # Contiguous DRAM chunk -> [128, FREE]: partition p
                    # holds rows r0 + K_SUB*p + k (k = 0..K_SUB-1).
                    nc.sync.dma_start(
                        out=t[:], in_=scores_ext[m, r0 : r0 + ROWS_PER_CHUNK, :]
                    )
                for k in range(K_SUB):
                    col = ROW0 + c * K_SUB + k
                    nc.vector.scalar_tensor_tensor(
                        t[:, k * S : (k + 1) * S],
                        t[:, k * S : (k + 1) * S],
                        bias_sb[:, col : col + 1],
                        colb_sb[:, m * S : (m + 1) * S],
                        mybir.AluOpType.subtract,
                        mybir.AluOpType.add,
                    )
                nc.scalar.dma_start(
                    out=out_ext[m, r0 : r0 + ROWS_PER_CHUNK, :], in_=t[:]
                )
    nc.compile()
    return nc


def _make_in_maps(scores, positions, token_indices, mode="pebcast3"):
    scores = np.ascontiguousarray(np.asarray(scores, dtype=np.float32))
    positions = np.asarray(positions, dtype=np.float32)
    tidx = np.asarray(token_indices).astype(np.int64)

    # slopes: match reference's f32 computation
    slopes = np.exp2((-8.0 * np.arange(1, H + 1) / H).astype(np.float32)).astype(
        np.float64
    )
    pos = positions.astype(np.float64)[tidx]  # [B, S]

    scores_flat = scores.reshape(B * H, S, S)
    p = np.arange(128)

    in_maps = []
    for core in range(NCORES):
        ms = np.arange(core * M_PER_CORE, (core + 1) * M_PER_CORE)
        bs, hs = ms // H, ms % H
        # rowv[p, c*K_SUB + k] = slope_m * pos[b_m, r0 + K_SUB*p + k]
        rowv = np.empty((128, N_CHUNKS * K_SUB), dtype=np.float32)
        for c in range(N_CHUNKS):
            m_loc = c // CHUNKS_PER_MAT
            r0 = (c % CHUNKS_PER_MAT) * ROWS_PER_CHUNK
            for k in range(K_SUB):
                rows = r0 + K_SUB * p + k
                rowv[:, c * K_SUB + k] = slopes[hs[m_loc]] * pos[bs[m_loc], rows]
        colv = (slopes[hs][:, None] * pos[bs]).astype(np.float32)  # [M_PER_CORE, S]
        im = {"scores": scores_flat[core * M_PER_CORE : (core + 1) * M_PER_CORE]}
        if mode == "packed":
            # bias = [colb | rowv]; colb[p, m_loc*S + f] = slope_m * pos[b_m, f]
            bias = np.empty(
                (128, M_PER_CORE * S + N_CHUNKS * K_SUB), dtype=np.float32
            )
            bias[:, : M_PER_CORE * S] = colv.reshape(1, M_PER_CORE * S)
            bias[:, M_PER_CORE * S :] = rowv
        else:
            bias = rowv
            im["colv"] = colv.reshape(-1)
        im["bias"] = bias
        in_maps.append(im)
    return in_maps


def _run(scores, positions, token_indices, trace=False, reps=1, mode="pebcast3"):
    in_maps = _make_in_maps(scores, positions, token_indices, mode)
    nc = _build_graph(mode)
    res = run_bass_kernel_spmd(nc, in_maps, core_ids=list(range(NCORES)), trace=trace)
    times = [res.exec_time_ns]
    for _ in range(reps - 1):
        r2 = run_bass_kernel_spmd(
            nc, in_maps, core_ids=list(range(NCORES)), trace=trace
        )
        times.append(r2.exec_time_ns)
    outs = [res.results[i]["out"] for i in range(NCORES)]
    full = np.concatenate(outs, axis=0).reshape(B, H, S, S)
    return full, res, times


def kernel(scores, positions, token_indices):
    full, _, _ = _run(scores, positions, token_indices, trace=False)
    return full
